# revision 7
# baseline (speedup 1.0000x reference)
"""Trainium2 Bass kernel for NeuralInelasticModel (3-layer ReLU MLP fwd + analytic Jacobians).

Data-parallel over 8 NeuronCores: each core processes 8192 of the 65536
(ntime*nbatch) samples. Activations are kept feature-major on-chip
(features on SBUF partitions, samples on the free dim) so biases fuse into
the ACT relu and every matmul streams 512-sample tiles at full rate.

The Jacobian J = w3 @ diag(m2) @ w2 @ diag(m1) @ w1 is computed as
  B_i = (w3[i,:] * m2) @ w2   -> 6 dense matmuls with stationary W2i = w2 * w3[i,:,None]
  J_i = (B_i * m1) @ w1
All matmuls run in float32r (fp32 storage, FP22 multiply) for 4x PE rate.
"""

import os
import sys

for _p in ("/root/.axon_site", "/root/.axon_site/_ro/trn_rl_repo",
           "/root/.axon_site/_ro/pypackages", "/opt/trn_rl_repo", "/opt/pypackages"):
    if os.path.isdir(_p) and _p not in sys.path:
        sys.path.append(_p)

import numpy as np

N_CORES = 8
NT, NB = 64, 1024
S = NT * NB
SC = S // N_CORES          # samples per core
NS = 6                     # state size
NI = NS + 2                # input features
H = 256                    # hidden width
TILE = 512                 # samples per on-chip tile (one fp32 PSUM bank)
NTILES = SC // TILE

_PROG = None               # (nc, in_names) cache — build/compile once per process


def _build_program():
    import concourse.bacc as bacc
    import concourse.mybir as mybir
    from concourse.bass import ts
    from concourse.tile import TileContext

    f32 = mybir.dt.float32
    f32r = mybir.dt.float32r
    mult = mybir.AluOpType.mult
    is_gt = mybir.AluOpType.is_gt
    Relu = mybir.ActivationFunctionType.Relu
    Copy = mybir.ActivationFunctionType.Copy

    nc = bacc.Bacc("TRN2", target_bir_lowering=False, debug=False,
                   num_devices=N_CORES)

    xT_d = nc.dram_tensor("xT", [NI, SC], f32r, kind="ExternalInput")
    w1T_d = nc.dram_tensor("w1T", [NI, H], f32r, kind="ExternalInput")
    w1c_d = nc.dram_tensor("w1c", [128, 2, NI], f32r, kind="ExternalInput")
    w2Tc_d = nc.dram_tensor("w2Tc", [128, 2, H], f32r, kind="ExternalInput")
    w2c_d = nc.dram_tensor("w2c", [128, 2, H], f32, kind="ExternalInput")
    w3Tc_d = nc.dram_tensor("w3Tc", [128, 2, NS], f32r, kind="ExternalInput")
    b1c_d = nc.dram_tensor("b1c", [128, 2], f32, kind="ExternalInput")
    b2c_d = nc.dram_tensor("b2c", [128, 2], f32, kind="ExternalInput")

    ydotT_d = nc.dram_tensor("ydotT", [NS, SC], f32, kind="ExternalOutput")
    JT_d = nc.dram_tensor("JT", [NS * NI, SC], f32, kind="ExternalOutput")

    with TileContext(nc) as tc:
        with (tc.tile_pool(name="consts", bufs=1) as consts,
              tc.tile_pool(name="acts", bufs=2) as acts,
              tc.tile_pool(name="ypool", bufs=3) as ypool,
              tc.tile_pool(name="psb", bufs=3, space="PSUM") as psb,
              tc.tile_pool(name="pss", bufs=2, space="PSUM") as pss):
            w1T_sb = consts.tile([NI, H], f32r)
            nc.sync.dma_start(w1T_sb[:], w1T_d[:])
            w1_sb = consts.tile([128, 2, NI], f32r)
            nc.sync.dma_start(w1_sb[:], w1c_d[:])
            w2T_sb = consts.tile([128, 2, H], f32r)
            nc.sync.dma_start(w2T_sb[:], w2Tc_d[:])
            w2_sb = consts.tile([128, 2, H], f32)
            nc.sync.dma_start(w2_sb[:], w2c_d[:])
            w3T_sb = consts.tile([128, 2, NS], f32r)
            nc.sync.dma_start(w3T_sb[:], w3Tc_d[:])
            b1_sb = consts.tile([128, 2], f32)
            nc.sync.dma_start(b1_sb[:], b1c_d[:])
            b2_sb = consts.tile([128, 2], f32)
            nc.sync.dma_start(b2_sb[:], b2c_d[:])

            # W2i[:, i, k, :] = w2[k-chunk, :] * w3[i, k-chunk] (per-partition scalar)
            W2i_sb = consts.tile([128, NS, 2, H], f32r)
            for i in range(NS):
                for k in range(2):
                    nc.vector.tensor_scalar(
                        W2i_sb[:, i, k, :], w2_sb[:, k, :],
                        w3T_sb[:, k, i:i + 1].bitcast(f32), None, mult)

            for t in range(NTILES):
                sl = ts(t, TILE)
                xT_sb = acts.tile([NI, TILE], f32r, tag="xT")
                nc.sync.dma_start(xT_sb[:], xT_d[:, sl])

                # z1.T = w1 @ x.T   (K=8)
                z1p = psb.tile([128, 2, TILE], f32, tag="big")
                for c in range(2):
                    nc.tensor.matmul(z1p[:, c, :],
                                     lhsT=w1T_sb[:, ts(c, 128)],
                                     rhs=xT_sb[:],
                                     start=True, stop=True)
                v1_sb = acts.tile([128, 2, TILE], f32r, tag="v1")
                m1_sb = acts.tile([128, 2, TILE], f32, tag="m1")
                for c in range(2):
                    nc.scalar.activation(v1_sb[:, c, :], z1p[:, c, :], Relu,
                                         bias=b1_sb[:, c:c + 1])
                for c in range(2):
                    nc.vector.tensor_scalar(m1_sb[:, c, :], v1_sb[:, c, :],
                                            0.0, None, is_gt)

                # z2.T = w2 @ v1.T  (K=256)
                z2p = psb.tile([128, 2, TILE], f32, tag="big")
                for c in range(2):
                    for k in range(2):
                        nc.tensor.matmul(z2p[:, c, :],
                                         lhsT=w2T_sb[:, k, ts(c, 128)],
                                         rhs=v1_sb[:, k, :],
                                         start=(k == 0), stop=(k == 1))
                v2_sb = acts.tile([128, 2, TILE], f32r, tag="v2")
                m2_sb = acts.tile([128, 2, TILE], f32r, tag="m2")
                for c in range(2):
                    nc.scalar.activation(v2_sb[:, c, :], z2p[:, c, :], Relu,
                                         bias=b2_sb[:, c:c + 1])
                for c in range(2):
                    nc.vector.tensor_scalar(m2_sb[:, c, :], v2_sb[:, c, :],
                                            0.0, None, is_gt)

                # ydot.T = w3 @ v2.T (+ b3 added on host)
                ydp = pss.tile([NI, TILE], f32, tag="sm")
                for k in range(2):
                    nc.tensor.matmul(ydp[:NS, :],
                                     lhsT=w3T_sb[:, k, :],
                                     rhs=v2_sb[:, k, :],
                                     start=(k == 0), stop=(k == 1))
                yd_sb = acts.tile([NS, TILE], f32, tag="yd")
                nc.scalar.activation(yd_sb[:], ydp[:NS, :], Copy)
                nc.sync.dma_start(ydotT_d[:, sl], yd_sb[:])

                # B_i.T = W2i.T @ m2.T ; Y_i = B_i * m1 ; J_i.T = w1.T @ Y_i.T
                for i in range(NS):
                    bp = psb.tile([128, 2, TILE], f32, tag="big")
                    for c in range(2):
                        for k in range(2):
                            nc.tensor.matmul(
                                bp[:, c, :],
                                lhsT=W2i_sb[:, i, k, ts(c, 128)],
                                rhs=m2_sb[:, k, :],
                                start=(k == 0), stop=(k == 1))
                    yi = ypool.tile([128, 2, TILE], f32r, tag="Y")
                    nc.vector.tensor_tensor(yi[:], bp[:], m1_sb[:], mult)
                    jp = pss.tile([NI, TILE], f32, tag="sm")
                    for k in range(2):
                        nc.tensor.matmul(jp[:],
                                         lhsT=w1_sb[:, k, :],
                                         rhs=yi[:, k, :],
                                         start=(k == 0), stop=(k == 1))
                    j_sb = acts.tile([NI, TILE], f32, tag="jout")
                    nc.scalar.activation(j_sb[:], jp[:], Copy)
                    nc.sync.dma_start(JT_d[ts(i, NI), sl], j_sb[:])

    nc.compile()
    in_names = ["xT", "w1T", "w1c", "w2Tc", "w2c", "w3Tc", "b1c", "b2c"]
    return nc, in_names


def _get_program():
    global _PROG
    if _PROG is None:
        _PROG = _build_program()
    return _PROG


def _prep_inputs(t, y, erate, T, w1, w2, w3, b1, b2, b3):
    """Host-side layout prep. Returns (in_maps, b3)."""
    f = np.float32
    xT = np.empty((NI, S), dtype=f)
    xT[:NS] = y.reshape(S, NS).T
    xT[NS] = erate.reshape(S)
    xT[NS + 1] = T.reshape(S)

    def chunked(a):
        # (256, m) -> [128, 2, m] with h = c*128 + p
        return np.ascontiguousarray(
            a.reshape(2, 128, -1).transpose(1, 0, 2)).astype(f, copy=False)

    w1T = np.ascontiguousarray(w1.T, dtype=f)            # (8, 256)
    w1c = chunked(w1)                                    # [128, 2, 8]
    w2Tc = chunked(np.ascontiguousarray(w2.T))           # [128, 2, 256]
    w2c = chunked(w2)                                    # [128, 2, 256]
    w3Tc = chunked(np.ascontiguousarray(w3.T))           # [128, 2, 6]
    b1c = np.ascontiguousarray(b1.reshape(2, 128).T, dtype=f)   # [128, 2]
    b2c = np.ascontiguousarray(b2.reshape(2, 128).T, dtype=f)

    in_maps = []
    for c in range(N_CORES):
        in_maps.append({
            "xT": np.ascontiguousarray(xT[:, c * SC:(c + 1) * SC]),
            "w1T": w1T, "w1c": w1c, "w2Tc": w2Tc, "w2c": w2c,
            "w3Tc": w3Tc, "b1c": b1c, "b2c": b2c,
        })
    return in_maps, np.asarray(b3, dtype=f)


def _assemble(results, b3):
    """Per-core {ydotT, JT} -> full (ydot, dydot_dy, dydot_de, dydot_dT)."""
    f = np.float32
    ydot = np.empty((S, NS), dtype=f)
    J = np.empty((S, NS, NI), dtype=f)
    for c in range(N_CORES):
        sl = slice(c * SC, (c + 1) * SC)
        ydot[sl] = results[c]["ydotT"].T
        J[sl] = results[c]["JT"].T.reshape(SC, NS, NI)
    ydot += b3
    ydot = ydot.reshape(NT, NB, NS)
    J = J.reshape(NT, NB, NS, NI)
    return (ydot,
            np.ascontiguousarray(J[..., :NS]),
            np.ascontiguousarray(J[..., NS]),
            np.ascontiguousarray(J[..., NS + 1]))


def kernel(t, y, erate, T, w1, w2, w3, b1, b2, b3):
    from concourse.bass_utils import run_bass_kernel_spmd

    nc, _ = _get_program()
    in_maps, b3 = _prep_inputs(t, y, erate, T, w1, w2, w3, b1, b2, b3)
    res = run_bass_kernel_spmd(nc, in_maps, list(range(N_CORES)))
    return _assemble(res.results, b3)


# revision 8
# speedup vs baseline: 884.7698x; 884.7698x over previous
"""Trainium2 Bass kernel for NeuralInelasticModel (3-layer ReLU MLP fwd + analytic Jacobians).

Data-parallel over 8 NeuronCores: each core processes 8192 of the 65536
(ntime*nbatch) samples. Activations are kept feature-major on-chip
(features on SBUF partitions, samples on the free dim) so biases fuse into
the ACT relu and every matmul streams 512-sample tiles at full rate.

The Jacobian J = w3 @ diag(m2) @ w2 @ diag(m1) @ w1 is computed as
  B_i = (w3[i,:] * m2) @ w2   -> 6 dense matmuls with stationary W2i = w2 * w3[i,:,None]
  J_i = (B_i * m1) @ w1
z1/z2 run in true fp32 so the ReLU masks match the fp32 reference (mask flips near z=0 dominate Jacobian error otherwise); the Jacobian
matmuls run in float32r (fp32 storage, ~13-bit multiply) for 4x PE rate.
"""

import os
import sys

for _p in ("/root/.axon_site", "/root/.axon_site/_ro/trn_rl_repo",
           "/root/.axon_site/_ro/pypackages", "/opt/trn_rl_repo", "/opt/pypackages"):
    if os.path.isdir(_p) and _p not in sys.path:
        sys.path.append(_p)

import numpy as np

N_CORES = 8
NT, NB = 64, 1024
S = NT * NB
SC = S // N_CORES          # samples per core
NS = 6                     # state size
NI = NS + 2                # input features
H = 256                    # hidden width
TILE = 512                 # samples per on-chip tile (one fp32 PSUM bank)
NTILES = SC // TILE

_PROG = None               # (nc, in_names) cache — build/compile once per process


def _build_program():
    import concourse.bacc as bacc
    import concourse.mybir as mybir
    from concourse.bass import ts
    from concourse.tile import TileContext

    f32 = mybir.dt.float32
    f32r = mybir.dt.float32r
    mult = mybir.AluOpType.mult
    is_gt = mybir.AluOpType.is_gt
    Relu = mybir.ActivationFunctionType.Relu
    Copy = mybir.ActivationFunctionType.Copy

    nc = bacc.Bacc("TRN2", target_bir_lowering=False, debug=False,
                   num_devices=N_CORES)

    xT_d = nc.dram_tensor("xT", [NI, SC], f32, kind="ExternalInput")
    w1T_d = nc.dram_tensor("w1T", [NI, H], f32, kind="ExternalInput")
    w1c_d = nc.dram_tensor("w1c", [128, 2, NI], f32r, kind="ExternalInput")
    w2Tc_d = nc.dram_tensor("w2Tc", [128, 2, H], f32, kind="ExternalInput")
    w2c_d = nc.dram_tensor("w2c", [128, 2, H], f32, kind="ExternalInput")
    w3Tc_d = nc.dram_tensor("w3Tc", [128, 2, NS], f32r, kind="ExternalInput")
    b1c_d = nc.dram_tensor("b1c", [128, 2], f32, kind="ExternalInput")
    b2c_d = nc.dram_tensor("b2c", [128, 2], f32, kind="ExternalInput")

    ydotT_d = nc.dram_tensor("ydotT", [NS, SC], f32, kind="ExternalOutput")
    JT_d = nc.dram_tensor("JT", [NS * NI, SC], f32, kind="ExternalOutput")

    with TileContext(nc) as tc:
        with (tc.tile_pool(name="consts", bufs=1) as consts,
              tc.tile_pool(name="acts", bufs=2) as acts,
              tc.tile_pool(name="ypool", bufs=3) as ypool,
              tc.tile_pool(name="psb", bufs=3, space="PSUM") as psb,
              tc.tile_pool(name="pss", bufs=2, space="PSUM") as pss):
            w1T_sb = consts.tile([NI, H], f32)
            nc.sync.dma_start(w1T_sb[:], w1T_d[:])
            w1_sb = consts.tile([128, 2, NI], f32r)
            nc.sync.dma_start(w1_sb[:], w1c_d[:])
            w2T_sb = consts.tile([128, 2, H], f32)
            nc.sync.dma_start(w2T_sb[:], w2Tc_d[:])
            w2_sb = consts.tile([128, 2, H], f32)
            nc.sync.dma_start(w2_sb[:], w2c_d[:])
            w3T_sb = consts.tile([128, 2, NS], f32r)
            nc.sync.dma_start(w3T_sb[:], w3Tc_d[:])
            b1_sb = consts.tile([128, 2], f32)
            nc.sync.dma_start(b1_sb[:], b1c_d[:])
            b2_sb = consts.tile([128, 2], f32)
            nc.sync.dma_start(b2_sb[:], b2c_d[:])

            # W2i[:, i, k, :] = w2[k-chunk, :] * w3[i, k-chunk] (per-partition scalar)
            W2i_sb = consts.tile([128, NS, 2, H], f32r)
            for i in range(NS):
                for k in range(2):
                    nc.vector.tensor_scalar(
                        W2i_sb[:, i, k, :], w2_sb[:, k, :],
                        w3T_sb[:, k, i:i + 1].bitcast(f32), None, mult)

            for t in range(NTILES):
                sl = ts(t, TILE)
                xT_sb = acts.tile([NI, TILE], f32, tag="xT")
                nc.sync.dma_start(xT_sb[:], xT_d[:, sl])

                # z1.T = w1 @ x.T   (K=8)
                z1p = psb.tile([128, 2, TILE], f32, tag="big")
                for c in range(2):
                    nc.tensor.matmul(z1p[:, c, :],
                                     lhsT=w1T_sb[:, ts(c, 128)],
                                     rhs=xT_sb[:],
                                     start=True, stop=True)
                v1_sb = acts.tile([128, 2, TILE], f32, tag="v1")
                m1_sb = acts.tile([128, 2, TILE], f32, tag="m1")
                for c in range(2):
                    nc.scalar.activation(v1_sb[:, c, :], z1p[:, c, :], Relu,
                                         bias=b1_sb[:, c:c + 1])
                for c in range(2):
                    nc.vector.tensor_scalar(m1_sb[:, c, :], v1_sb[:, c, :],
                                            0.0, None, is_gt)

                # z2.T = w2 @ v1.T  (K=256)
                z2p = psb.tile([128, 2, TILE], f32, tag="big")
                for c in range(2):
                    for k in range(2):
                        nc.tensor.matmul(z2p[:, c, :],
                                         lhsT=w2T_sb[:, k, ts(c, 128)],
                                         rhs=v1_sb[:, k, :],
                                         start=(k == 0), stop=(k == 1))
                v2_sb = acts.tile([128, 2, TILE], f32r, tag="v2")
                m2_sb = acts.tile([128, 2, TILE], f32r, tag="m2")
                for c in range(2):
                    nc.scalar.activation(v2_sb[:, c, :], z2p[:, c, :], Relu,
                                         bias=b2_sb[:, c:c + 1])
                for c in range(2):
                    nc.vector.tensor_scalar(m2_sb[:, c, :], v2_sb[:, c, :],
                                            0.0, None, is_gt)

                # ydot.T = w3 @ v2.T (+ b3 added on host)
                ydp = pss.tile([NI, TILE], f32, tag="sm")
                for k in range(2):
                    nc.tensor.matmul(ydp[:NS, :],
                                     lhsT=w3T_sb[:, k, :],
                                     rhs=v2_sb[:, k, :],
                                     start=(k == 0), stop=(k == 1))
                yd_sb = acts.tile([NS, TILE], f32, tag="yd")
                nc.scalar.activation(yd_sb[:], ydp[:NS, :], Copy)
                nc.sync.dma_start(ydotT_d[:, sl], yd_sb[:])

                # B_i.T = W2i.T @ m2.T ; Y_i = B_i * m1 ; J_i.T = w1.T @ Y_i.T
                for i in range(NS):
                    bp = psb.tile([128, 2, TILE], f32, tag="big")
                    for c in range(2):
                        for k in range(2):
                            nc.tensor.matmul(
                                bp[:, c, :],
                                lhsT=W2i_sb[:, i, k, ts(c, 128)],
                                rhs=m2_sb[:, k, :],
                                start=(k == 0), stop=(k == 1))
                    yi = ypool.tile([128, 2, TILE], f32r, tag="Y")
                    nc.vector.tensor_tensor(yi[:], bp[:], m1_sb[:], mult)
                    jp = pss.tile([NI, TILE], f32, tag="sm")
                    for k in range(2):
                        nc.tensor.matmul(jp[:],
                                         lhsT=w1_sb[:, k, :],
                                         rhs=yi[:, k, :],
                                         start=(k == 0), stop=(k == 1))
                    j_sb = acts.tile([NI, TILE], f32, tag="jout")
                    nc.scalar.activation(j_sb[:], jp[:], Copy)
                    nc.sync.dma_start(JT_d[ts(i, NI), sl], j_sb[:])

    nc.compile()
    in_names = ["xT", "w1T", "w1c", "w2Tc", "w2c", "w3Tc", "b1c", "b2c"]
    return nc, in_names


def _get_program():
    global _PROG
    if _PROG is None:
        _PROG = _build_program()
    return _PROG


def _prep_inputs(t, y, erate, T, w1, w2, w3, b1, b2, b3):
    """Host-side layout prep. Returns (in_maps, b3)."""
    f = np.float32
    xT = np.empty((NI, S), dtype=f)
    xT[:NS] = y.reshape(S, NS).T
    xT[NS] = erate.reshape(S)
    xT[NS + 1] = T.reshape(S)

    def chunked(a):
        # (256, m) -> [128, 2, m] with h = c*128 + p
        return np.ascontiguousarray(
            a.reshape(2, 128, -1).transpose(1, 0, 2)).astype(f, copy=False)

    w1T = np.ascontiguousarray(w1.T, dtype=f)            # (8, 256)
    w1c = chunked(w1)                                    # [128, 2, 8]
    w2Tc = chunked(np.ascontiguousarray(w2.T))           # [128, 2, 256]
    w2c = chunked(w2)                                    # [128, 2, 256]
    w3Tc = chunked(np.ascontiguousarray(w3.T))           # [128, 2, 6]
    b1c = np.ascontiguousarray(b1.reshape(2, 128).T, dtype=f)   # [128, 2]
    b2c = np.ascontiguousarray(b2.reshape(2, 128).T, dtype=f)

    in_maps = []
    for c in range(N_CORES):
        in_maps.append({
            "xT": np.ascontiguousarray(xT[:, c * SC:(c + 1) * SC]),
            "w1T": w1T, "w1c": w1c, "w2Tc": w2Tc, "w2c": w2c,
            "w3Tc": w3Tc, "b1c": b1c, "b2c": b2c,
        })
    return in_maps, np.asarray(b3, dtype=f)


def _assemble(results, b3):
    """Per-core {ydotT, JT} -> full (ydot, dydot_dy, dydot_de, dydot_dT)."""
    f = np.float32
    ydot = np.empty((S, NS), dtype=f)
    J = np.empty((S, NS, NI), dtype=f)
    for c in range(N_CORES):
        sl = slice(c * SC, (c + 1) * SC)
        ydot[sl] = results[c]["ydotT"].T
        J[sl] = results[c]["JT"].T.reshape(SC, NS, NI)
    ydot += b3
    ydot = ydot.reshape(NT, NB, NS)
    J = J.reshape(NT, NB, NS, NI)
    return (ydot,
            np.ascontiguousarray(J[..., :NS]),
            np.ascontiguousarray(J[..., NS]),
            np.ascontiguousarray(J[..., NS + 1]))


def kernel(t, y, erate, T, w1, w2, w3, b1, b2, b3):
    from concourse.bass_utils import run_bass_kernel_spmd

    nc, _ = _get_program()
    in_maps, b3 = _prep_inputs(t, y, erate, T, w1, w2, w3, b1, b2, b3)
    res = run_bass_kernel_spmd(nc, in_maps, list(range(N_CORES)))
    return _assemble(res.results, b3)


# revision 10
# speedup vs baseline: 4626.8151x; 5.2294x over previous
"""Trainium2 Bass kernel for NeuralInelasticModel (3-layer ReLU MLP fwd + analytic Jacobians).

Data-parallel over 8 NeuronCores: each core processes 8192 of the 65536
(ntime*nbatch) samples. Activations are kept feature-major on-chip
(features on SBUF partitions, samples on the free dim) so biases fuse into
the ACT relu and every matmul streams 512-sample tiles at full rate.

The Jacobian J = w3 @ diag(m2) @ w2 @ diag(m1) @ w1 is computed as
  B_i = (w3[i,:] * m2) @ w2   -> 6 dense matmuls with stationary W2i = w2 * w3[i,:,None]
  J_i = (B_i * m1) @ w1
z1/z2 run in true fp32 so the ReLU masks match the fp32 reference (mask flips near z=0 dominate Jacobian error otherwise); the Jacobian
matmuls run in float32r (fp32 storage, ~13-bit multiply) for 4x PE rate.
"""

import os
import sys

for _p in ("/root/.axon_site", "/root/.axon_site/_ro/trn_rl_repo",
           "/root/.axon_site/_ro/pypackages", "/opt/trn_rl_repo", "/opt/pypackages"):
    if os.path.isdir(_p) and _p not in sys.path:
        sys.path.append(_p)

import numpy as np

N_CORES = 8
NT, NB = 64, 1024
S = NT * NB
SC = S // N_CORES          # samples per core
NS = 6                     # state size
NI = NS + 2                # input features
H = 256                    # hidden width
TILE = 512                 # samples per on-chip tile (one fp32 PSUM bank)
NTILES = SC // TILE

_PROG = None               # (nc, in_names) cache — build/compile once per process


def _build_program(passes=1):
    """passes>1 repeats the whole computation (same outputs) for timing."""
    import concourse.bacc as bacc
    import concourse.mybir as mybir
    from concourse.bass import ts
    from concourse.tile import TileContext

    f32 = mybir.dt.float32
    f32r = mybir.dt.float32r
    mult = mybir.AluOpType.mult
    is_gt = mybir.AluOpType.is_gt
    Relu = mybir.ActivationFunctionType.Relu
    Copy = mybir.ActivationFunctionType.Copy

    nc = bacc.Bacc("TRN2", target_bir_lowering=False, debug=False,
                   num_devices=N_CORES)

    xT_d = nc.dram_tensor("xT", [NI, SC], f32, kind="ExternalInput")
    w1T_d = nc.dram_tensor("w1T", [NI, H], f32, kind="ExternalInput")
    w1c_d = nc.dram_tensor("w1c", [128, 2, NI], f32r, kind="ExternalInput")
    w2Tc_d = nc.dram_tensor("w2Tc", [128, 2, H], f32, kind="ExternalInput")
    w2c_d = nc.dram_tensor("w2c", [128, 2, H], f32, kind="ExternalInput")
    w3Tc_d = nc.dram_tensor("w3Tc", [128, 2, NS], f32r, kind="ExternalInput")
    b1c_d = nc.dram_tensor("b1c", [128, 2], f32, kind="ExternalInput")
    b2c_d = nc.dram_tensor("b2c", [128, 2], f32, kind="ExternalInput")

    ydotT_d = nc.dram_tensor("ydotT", [NS, SC], f32, kind="ExternalOutput")
    JT_d = nc.dram_tensor("JT", [NS * NI, SC], f32, kind="ExternalOutput")

    with TileContext(nc) as tc:
        with (tc.tile_pool(name="consts", bufs=1) as consts,
              tc.tile_pool(name="acts", bufs=2) as acts,
              tc.tile_pool(name="ypool", bufs=3) as ypool,
              tc.tile_pool(name="psb", bufs=3, space="PSUM") as psb,
              tc.tile_pool(name="pss", bufs=2, space="PSUM") as pss):
            w1T_sb = consts.tile([NI, H], f32)
            nc.sync.dma_start(w1T_sb[:], w1T_d[:])
            w1_sb = consts.tile([128, 2, NI], f32r)
            nc.sync.dma_start(w1_sb[:], w1c_d[:])
            w2T_sb = consts.tile([128, 2, H], f32)
            nc.sync.dma_start(w2T_sb[:], w2Tc_d[:])
            w2_sb = consts.tile([128, 2, H], f32)
            nc.sync.dma_start(w2_sb[:], w2c_d[:])
            w3T_sb = consts.tile([128, 2, NS], f32r)
            nc.sync.dma_start(w3T_sb[:], w3Tc_d[:])
            b1_sb = consts.tile([128, 2], f32)
            nc.sync.dma_start(b1_sb[:], b1c_d[:])
            b2_sb = consts.tile([128, 2], f32)
            nc.sync.dma_start(b2_sb[:], b2c_d[:])

            # W2i[:, i, k, :] = w2[k-chunk, :] * w3[i, k-chunk] (per-partition scalar)
            W2i_sb = consts.tile([128, NS, 2, H], f32r)
            for i in range(NS):
                for k in range(2):
                    nc.vector.tensor_scalar(
                        W2i_sb[:, i, k, :], w2_sb[:, k, :],
                        w3T_sb[:, k, i:i + 1].bitcast(f32), None, mult)

            for t in range(NTILES * passes):
                t = t % NTILES
                sl = ts(t, TILE)
                xT_sb = acts.tile([NI, TILE], f32, tag="xT")
                nc.sync.dma_start(xT_sb[:], xT_d[:, sl])

                # z1.T = w1 @ x.T   (K=8)
                z1p = psb.tile([128, 2, TILE], f32, tag="big")
                for c in range(2):
                    nc.tensor.matmul(z1p[:, c, :],
                                     lhsT=w1T_sb[:, ts(c, 128)],
                                     rhs=xT_sb[:],
                                     start=True, stop=True)
                v1_sb = acts.tile([128, 2, TILE], f32, tag="v1")
                m1_sb = acts.tile([128, 2, TILE], f32, tag="m1")
                for c in range(2):
                    nc.scalar.activation(v1_sb[:, c, :], z1p[:, c, :], Relu,
                                         bias=b1_sb[:, c:c + 1])
                for c in range(2):
                    nc.vector.tensor_scalar(m1_sb[:, c, :], v1_sb[:, c, :],
                                            0.0, None, is_gt)

                # z2.T = w2 @ v1.T  (K=256)
                z2p = psb.tile([128, 2, TILE], f32, tag="big")
                for c in range(2):
                    for k in range(2):
                        nc.tensor.matmul(z2p[:, c, :],
                                         lhsT=w2T_sb[:, k, ts(c, 128)],
                                         rhs=v1_sb[:, k, :],
                                         start=(k == 0), stop=(k == 1))
                v2_sb = acts.tile([128, 2, TILE], f32r, tag="v2")
                m2_sb = acts.tile([128, 2, TILE], f32r, tag="m2")
                for c in range(2):
                    nc.scalar.activation(v2_sb[:, c, :], z2p[:, c, :], Relu,
                                         bias=b2_sb[:, c:c + 1])
                for c in range(2):
                    nc.vector.tensor_scalar(m2_sb[:, c, :], v2_sb[:, c, :],
                                            0.0, None, is_gt)

                # ydot.T = w3 @ v2.T (+ b3 added on host)
                ydp = pss.tile([NI, TILE], f32, tag="sm")
                for k in range(2):
                    nc.tensor.matmul(ydp[:NS, :],
                                     lhsT=w3T_sb[:, k, :],
                                     rhs=v2_sb[:, k, :],
                                     start=(k == 0), stop=(k == 1))
                yd_sb = acts.tile([NS, TILE], f32, tag="yd")
                nc.scalar.activation(yd_sb[:], ydp[:NS, :], Copy)
                nc.sync.dma_start(ydotT_d[:, sl], yd_sb[:])

                # B_i.T = W2i.T @ m2.T ; Y_i = B_i * m1 ; J_i.T = w1.T @ Y_i.T
                for i in range(NS):
                    bp = psb.tile([128, 2, TILE], f32, tag="big")
                    for c in range(2):
                        for k in range(2):
                            nc.tensor.matmul(
                                bp[:, c, :],
                                lhsT=W2i_sb[:, i, k, ts(c, 128)],
                                rhs=m2_sb[:, k, :],
                                start=(k == 0), stop=(k == 1))
                    yi = ypool.tile([128, 2, TILE], f32r, tag="Y")
                    nc.vector.tensor_tensor(yi[:], bp[:], m1_sb[:], mult)
                    jp = pss.tile([NI, TILE], f32, tag="sm")
                    for k in range(2):
                        nc.tensor.matmul(jp[:],
                                         lhsT=w1_sb[:, k, :],
                                         rhs=yi[:, k, :],
                                         start=(k == 0), stop=(k == 1))
                    j_sb = acts.tile([NI, TILE], f32, tag="jout")
                    nc.scalar.activation(j_sb[:], jp[:], Copy)
                    nc.sync.dma_start(JT_d[ts(i, NI), sl], j_sb[:])

    nc.compile()
    in_names = ["xT", "w1T", "w1c", "w2Tc", "w2c", "w3Tc", "b1c", "b2c"]
    return nc, in_names


def _get_program():
    global _PROG
    if _PROG is None:
        _PROG = _build_program()
    return _PROG


def _prep_inputs(t, y, erate, T, w1, w2, w3, b1, b2, b3):
    """Host-side layout prep. Returns (in_maps, b3)."""
    f = np.float32
    xT = np.empty((NI, S), dtype=f)
    xT[:NS] = y.reshape(S, NS).T
    xT[NS] = erate.reshape(S)
    xT[NS + 1] = T.reshape(S)

    def chunked(a):
        # (256, m) -> [128, 2, m] with h = c*128 + p
        return np.ascontiguousarray(
            a.reshape(2, 128, -1).transpose(1, 0, 2)).astype(f, copy=False)

    w1T = np.ascontiguousarray(w1.T, dtype=f)            # (8, 256)
    w1c = chunked(w1)                                    # [128, 2, 8]
    w2Tc = chunked(np.ascontiguousarray(w2.T))           # [128, 2, 256]
    w2c = chunked(w2)                                    # [128, 2, 256]
    w3Tc = chunked(np.ascontiguousarray(w3.T))           # [128, 2, 6]
    b1c = np.ascontiguousarray(b1.reshape(2, 128).T, dtype=f)   # [128, 2]
    b2c = np.ascontiguousarray(b2.reshape(2, 128).T, dtype=f)

    in_maps = []
    for c in range(N_CORES):
        in_maps.append({
            "xT": np.ascontiguousarray(xT[:, c * SC:(c + 1) * SC]),
            "w1T": w1T, "w1c": w1c, "w2Tc": w2Tc, "w2c": w2c,
            "w3Tc": w3Tc, "b1c": b1c, "b2c": b2c,
        })
    return in_maps, np.asarray(b3, dtype=f)


def _assemble(results, b3):
    """Per-core {ydotT, JT} -> full (ydot, dydot_dy, dydot_de, dydot_dT)."""
    f = np.float32
    ydot = np.empty((S, NS), dtype=f)
    J = np.empty((S, NS, NI), dtype=f)
    for c in range(N_CORES):
        sl = slice(c * SC, (c + 1) * SC)
        ydot[sl] = results[c]["ydotT"].T
        J[sl] = results[c]["JT"].T.reshape(SC, NS, NI)
    ydot += b3
    ydot = ydot.reshape(NT, NB, NS)
    J = J.reshape(NT, NB, NS, NI)
    return (ydot,
            np.ascontiguousarray(J[..., :NS]),
            np.ascontiguousarray(J[..., NS]),
            np.ascontiguousarray(J[..., NS + 1]))


def kernel(t, y, erate, T, w1, w2, w3, b1, b2, b3):
    from concourse.bass_utils import run_bass_kernel_spmd

    nc, _ = _get_program()
    in_maps, b3 = _prep_inputs(t, y, erate, T, w1, w2, w3, b1, b2, b3)
    res = run_bass_kernel_spmd(nc, in_maps, list(range(N_CORES)))
    return _assemble(res.results, b3)
